# revision 1
# baseline (speedup 1.0000x reference)
"""Trainium2 Bass kernel for a custom attention block (qkv-proj + LN(q,k) +
RoPE + causal attention + out-proj), distributed over 8 NeuronCores.

Sharding: 2 cores per batch (B=4). Core role r=c%2 takes q-token blocks
{0,3} (r=0) or {1,2} (r=1) of 512 tokens; every core computes K/V for the
full 2048-token sequence of its batch (no collectives). The compiled
program is identical on all cores; all per-core differences are input
data (sliced x^T, cos/sin tables, causal masks).

Orientation: q^T / k^T are produced feature-on-partition ([hd, tokens]),
v token-on-partition. Attention computes s^T = (k^T)^T-slice @ q^T with
fp32r matmuls (full PE rate at moving-dim >= 256), exp(s - 8), mask
multiply (data-driven), PV as lhsT=v rhs=exp -> out^T, denominator via a
ones-column matmul, so no on-chip transposes are needed anywhere.

LN: mean subtraction is folded into host-pre-centered w_in rows; variance
comes from Square + ones-matmul partition reduction; rsqrt(var+eps) is
computed as Exp(-0.5*Ln(var+eps)) so all ACT functions live in one table
set (natural_log_exp_and_others).
"""

import math

import numpy as np

import concourse.bass as bass
import concourse.mybir as mybir
import concourse.tile as tile
from concourse import bacc
from concourse.bass import ds, ts

F32 = mybir.dt.float32
F32R = mybir.dt.float32r
AF = mybir.ActivationFunctionType
OP = mybir.AluOpType

P = 128
HD = 128

FULL_CFG = dict(
    D=2048,           # model dim (contraction dim for projections)
    S=2048,           # kv tokens per core (full sequence of its batch)
    NQTOK=1024,       # q tokens per core
    PT=256,           # projection s-tile width (moving dim)
    QT=512,           # attention q-tile width (moving dim)
    slots=(8, 16),    # kv 128-chunks visited per q-tile
    masked=(tuple(range(0, 8)), tuple(range(8, 16))),  # slots that get a mask
    EXP_BIAS=8.0,
    EPS=1e-5,
    MASK_F32=False,
)

SMALL_CFG = dict(
    D=512,
    S=1024,
    NQTOK=1024,
    PT=256,
    QT=512,
    slots=(8, 8),
    masked=(tuple(range(0, 8)), tuple(range(4, 8))),
    EXP_BIAS=8.0,
    EPS=1e-5,
    MASK_F32=False,
)


def _r(ap):
    """fp32 -> fp32r view for matmul operands."""
    return ap.bitcast(F32R)


def build_program(cfg):
    D = cfg["D"]
    S = cfg["S"]
    NQTOK = cfg["NQTOK"]
    PT = cfg["PT"]
    QT = cfg["QT"]
    slots = cfg["slots"]
    masked = cfg["masked"]
    EXP_BIAS = cfg["EXP_BIAS"]
    EPS = cfg["EPS"]

    NH = D // HD              # heads == e-chunks per q (and per k)
    DC = D // P               # contraction chunks
    NQ = NQTOK // QT          # q tiles
    S2 = S // 2               # kv half (x residency granularity)
    KC2 = S2 // P             # kv chunks per half
    VET = max(1, D // 512)    # v e-tiles of width 512
    VEW = min(512, D)         # v e-tile width
    VH = VEW // HD            # heads per v e-tile
    MAXM = max(len(m) for m in masked)
    QST_PER_TILE = QT // PT
    mdt = F32 if cfg.get("MASK_F32", True) else mybir.dt.bfloat16

    nc = bacc.Bacc("TRN2", target_bir_lowering=False, debug=False)

    # ---- I/O ----
    xTq = nc.dram_tensor("xTq", [D, NQTOK], F32, kind="ExternalInput").ap()
    xT = nc.dram_tensor("xT", [D, S], F32, kind="ExternalInput").ap()
    wqkT = nc.dram_tensor("wqkT", [2 * NH, P, DC, P], F32,
                          kind="ExternalInput").ap()
    wvT = nc.dram_tensor("wvT", [D, D], F32, kind="ExternalInput").ap()
    woT = nc.dram_tensor("woT", [D, D], F32, kind="ExternalInput").ap()
    cosq_i = nc.dram_tensor("cosq", [HD, NQTOK], F32, kind="ExternalInput").ap()
    sinq_i = nc.dram_tensor("sinqn", [HD, NQTOK], F32, kind="ExternalInput").ap()
    cosk_i = nc.dram_tensor("cosk", [HD, S], F32, kind="ExternalInput").ap()
    sink_i = nc.dram_tensor("sinkn", [HD, S], F32, kind="ExternalInput").ap()
    rotm_i = nc.dram_tensor("rotm", [P, P], F32, kind="ExternalInput").ap()
    onesc_i = nc.dram_tensor("onesc", [P, 1], F32, kind="ExternalInput").ap()
    onesr_i = nc.dram_tensor("onesr", [1, P], F32, kind="ExternalInput").ap()
    gq_i = nc.dram_tensor("gq", [P, NH], F32, kind="ExternalInput").ap()
    bq_i = nc.dram_tensor("bq", [P, NH], F32, kind="ExternalInput").ap()
    gk_i = nc.dram_tensor("gk", [P, NH], F32, kind="ExternalInput").ap()
    bk_i = nc.dram_tensor("bk", [P, NH], F32, kind="ExternalInput").ap()
    masks_i = nc.dram_tensor("masks", [NQ, P, MAXM, QT], mdt, kind="ExternalInput").ap()
    out_t = nc.dram_tensor("out", [D, NQTOK], F32, kind="ExternalOutput").ap()

    with tile.TileContext(nc) as tc:
        import contextlib

        ctx = contextlib.ExitStack()
        with ctx:
            sb = ctx.enter_context(tc.tile_pool(name="sb", bufs=1))
            psum = ctx.enter_context(tc.tile_pool(name="ps", bufs=1, space="PSUM"))
            dram = ctx.enter_context(tc.tile_pool(name="dram", bufs=1, space="DRAM"))

            # ---- DRAM scratch ----
            qts = dram.tile([P, NH, NQTOK], F32, tag="qts", name="qts")
            kts = dram.tile([P, NH, S], F32, tag="kts", name="kts")
            vs = dram.tile([NH, S, HD], F32, tag="vs", name="vs")
            ots = dram.tile([P, NH, NQTOK], F32, tag="ots", name="ots")

            # ---- constants / small inputs ----
            ones_col = sb.tile([P, 1], F32, tag="ones_col", name="ones_col")
            nc.sync.dma_start(_r(ones_col), _r(onesc_i))
            ones_row = sb.tile([1, P], F32, tag="ones_row", name="ones_row")
            nc.sync.dma_start(_r(ones_row), _r(onesr_i))
            eps1 = sb.tile([1, 1], F32, tag="eps1", name="eps1")
            nc.vector.memset(eps1, EPS)
            zero1 = sb.tile([1, 1], F32, tag="zero1", name="zero1")
            nc.vector.memset(zero1, 0.0)
            nege = sb.tile([P, 1], F32, tag="nege", name="nege")
            nc.vector.memset(nege, -EXP_BIAS)
            rotm = sb.tile([P, P], F32, tag="rotm", name="rotm")
            nc.sync.dma_start(_r(rotm), _r(rotm_i))
            gq = sb.tile([P, NH], F32, tag="gq", name="gq")
            nc.sync.dma_start(gq, gq_i)
            bq = sb.tile([P, NH], F32, tag="bq", name="bq")
            nc.sync.dma_start(bq, bq_i)
            gk = sb.tile([P, NH], F32, tag="gk", name="gk")
            nc.sync.dma_start(gk, gk_i)
            bk = sb.tile([P, NH], F32, tag="bk", name="bk")
            nc.sync.dma_start(bk, bk_i)

            def proj_ln_rope(x_sb, n_tok, st_global_off, wcol_off, n_st,
                             cos_sb, sin_sb, g_sb, b_sb, dst, tok0):
                """Project x_sb -> feature-partition [e, s] tiles, LN, rope,
                write to dst[:, :, tok0 + st*PT ...].

                x_sb: [P, DC, n_tok] sbuf; st covers n_st tiles of PT inside.
                wcol_off: column offset into wqkT (0 for q, D for k).
                cos_sb/sin_sb indexed at st_global_off + local offsets.
                """
                assert n_st % 2 == 0
                for grp in range(n_st // 2):
                    sts = [grp * 2, grp * 2 + 1]
                    gsl = ds(st_global_off + grp * 2 * PT, 2 * PT)
                    cos_t = sb.tile([HD, 2 * PT], F32, tag="cos", bufs=2,
                                    name="cos_t")
                    nc.sync.dma_start(cos_t, cos_sb[:, gsl])
                    sin_t = sb.tile([HD, 2 * PT], F32, tag="sin", bufs=2,
                                    name="sin_t")
                    nc.sync.dma_start(sin_t, sin_sb[:, gsl])
                    holds = {}
                    pstats = {}
                    for st in sts:
                        holds[st] = sb.tile([P, NH, PT], F32, tag="hold",
                                            bufs=3, name="hold")
                        pstats[st] = psum.tile([1, PT], F32, tag="stat",
                                               bufs=4, name="ps_stat")
                    for ec in range(NH):
                        w = sb.tile([P, DC, P], F32, tag="w", bufs=3, name="w")
                        nc.sync.dma_start(
                            _r(w), _r(wqkT[wcol_off // P + ec])
                        )
                        pss = {st: psum.tile([P, PT], F32, tag="mm", bufs=4,
                                             name="ps")
                               for st in sts}
                        for d in range(DC):
                            for st in sts:
                                nc.tensor.matmul(
                                    pss[st],
                                    lhsT=_r(w[:, d]),
                                    rhs=_r(x_sb[:, d, ds(st * PT, PT)]),
                                    start=(d == 0),
                                    stop=(d == DC - 1),
                                )
                        for st in sts:
                            nc.vector.tensor_copy(_r(holds[st][:, ec]), pss[st])
                            sq = sb.tile([P, PT], F32, tag="sq", bufs=2,
                                         name="sq")
                            nc.scalar.square(_r(sq), pss[st])
                            nc.tensor.matmul(
                                pstats[st],
                                lhsT=_r(ones_col),
                                rhs=_r(sq),
                                start=(ec == 0),
                                stop=(ec == NH - 1),
                            )
                    for st in sts:
                        hold = holds[st]
                        csl = ds((st % 2) * PT, PT)
                        # rsig = exp(-0.5 * ln(sumsq/D + eps))
                        lnv = sb.tile([1, PT], F32, tag="stats_sb", bufs=4,
                                      name="lnv")
                        nc.scalar.activation(lnv, pstats[st], AF.Ln,
                                             scale=1.0 / D, bias=eps1)
                        rsig = sb.tile([1, PT], F32, tag="stats_sb", bufs=4,
                                       name="rsig")
                        nc.scalar.activation(_r(rsig), lnv, AF.Exp, bias=zero1,
                                             scale=-0.5)
                        ps_rep = psum.tile([P, PT], F32, tag="stat", bufs=4,
                                           name="ps_rep")
                        nc.tensor.matmul(ps_rep, lhsT=_r(ones_row),
                                         rhs=_r(rsig))
                        # pass 1: DVE LN apply for all chunks first, so the
                        # rotation matmuls never head-of-line block the
                        # in-order PE stream on a DVE dependency.
                        for ec in range(NH):
                            ch = hold[:, ec]
                            nc.vector.tensor_tensor(_r(ch), ch, ps_rep,
                                                    op=OP.mult)
                            nc.vector.tensor_scalar(
                                _r(ch), ch,
                                scalar1=g_sb[:, ds(ec, 1)],
                                scalar2=b_sb[:, ds(ec, 1)],
                                op0=OP.mult, op1=OP.add,
                            )
                        # pass 2: rotation matmuls stream back-to-back
                        for ec in range(NH):
                            ch = hold[:, ec]
                            ps_rot = psum.tile([P, PT], F32, tag="mm", bufs=4,
                                               name="ps_rot")
                            nc.tensor.matmul(ps_rot, lhsT=_r(rotm), rhs=_r(ch))
                            tmp = sb.tile([P, PT], F32, tag="tmp", bufs=3,
                                          name="rtmp")
                            nc.vector.tensor_tensor(
                                tmp, ps_rot, sin_t[:, csl], op=OP.mult
                            )
                            nc.vector.tensor_tensor(_r(ch), ch, cos_t[:, csl],
                                                    op=OP.mult)
                            nc.vector.tensor_tensor(_r(ch), ch, tmp, op=OP.add)
                        nc.sync.dma_start(dst[:, :, ds(tok0 + st * PT, PT)],
                                          hold)

            # ---- Phase A: q projection ----
            xq = sb.tile([P, DC, max(NQTOK, S2)], F32, tag="bigx", bufs=1,
                         name="xq")
            xq = xq[:, :, :NQTOK]
            for d in range(DC):
                nc.sync.dma_start(_r(xq[:, d]), _r(xTq[ds(d * P, P), :]))
            proj_ln_rope(xq, NQTOK, 0, 0, NQTOK // PT, cosq_i, sinq_i,
                         gq, bq, qts, 0)

            # ---- Phase B+C: k and v projections, per x-half ----
            for half in range(2):
                xk = sb.tile([P, DC, max(NQTOK, S2)], F32, tag="bigx", bufs=1,
                             name="xk")
                xk = xk[:, :, :S2]
                for d in range(DC):
                    nc.sync.dma_start(
                        _r(xk[:, d]), _r(xT[ds(d * P, P), ds(half * S2, S2)])
                    )
                proj_ln_rope(xk, S2, half * S2, D, S2 // PT, cosk_i, sink_i,
                             gk, bk, kts, half * S2)
                # v: natural orientation, x as stationary
                n_grp = (KC2 + 3) // 4
                for grp in range(n_grp):
                    scs = [sc for sc in range(grp * 4, min((grp + 1) * 4, KC2))]
                    for et in range(VET):
                        psv = {}
                        for sc in scs:
                            psv[sc] = psum.tile([P, VEW], F32, tag="mm",
                                                bufs=4, name="psv")
                        for d in range(DC):
                            wv = sb.tile([P, VEW], F32, tag="w", bufs=3,
                                         name="wv")
                            nc.sync.dma_start(
                                _r(wv), _r(wvT[ds(d * P, P), ds(et * VEW, VEW)])
                            )
                            for sc in scs:
                                nc.tensor.matmul(
                                    psv[sc],
                                    lhsT=_r(xk[:, d, ds(sc * P, P)]),
                                    rhs=_r(wv),
                                    start=(d == 0),
                                    stop=(d == DC - 1),
                                )
                        for sc in scs:
                            vsb = sb.tile([P, VEW], F32, tag="vsb", bufs=2,
                                          name="vsb")
                            nc.vector.tensor_copy(vsb, psv[sc])
                            gsc = half * KC2 + sc
                            for hh in range(VH):
                                nc.sync.dma_start(
                                    vs[et * VH + hh, ds(gsc * P, P), :],
                                    vsb[:, ds(hh * HD, HD)],
                                )

            # ---- Phase D: attention + normalization ----
            for t in range(NQ):
                qsl_off = t * QT
                mt = sb.tile([P, MAXM, QT], mdt, tag="masks", bufs=1,
                             name="mt")
                nc.sync.dma_start(mt, masks_i[t])
                mpos = {kc: i for i, kc in enumerate(masked[t])}
                n_slots = slots[t]
                n_half = (n_slots + KC2 - 1) // KC2  # halves needed
                for h in range(NH):
                    qsl = sb.tile([P, QT], F32, tag="qslab", bufs=2,
                                  name="qsl")
                    nc.sync.dma_start(_r(qsl), _r(qts[:, h, ds(qsl_off, QT)]))
                    ksl = {}
                    vsl = {}
                    for hf in range(n_half):
                        ksl[hf] = sb.tile([P, S2], F32, tag="kslab", bufs=2,
                                          name="ksl")
                        nc.sync.dma_start(_r(ksl[hf]), _r(kts[:, h, ds(hf * S2, S2)]))
                        vsl[hf] = sb.tile([P, KC2, HD], F32, tag="vslab",
                                          bufs=2, name="vsl")
                        nc.sync.dma_start(
                            _r(vsl[hf]),
                            _r(vs[h, ds(hf * S2, S2), :].rearrange(
                                "(kc p) hd -> p kc hd", p=P
                            )),
                        )
                    psout = psum.tile([P, QT], F32, tag="mm", bufs=4,
                                      name="psout")
                    psden = psum.tile([1, QT], F32, tag="stat", bufs=4,
                                      name="psden")
                    for slot in range(n_slots):
                        hf = slot // KC2
                        kc = slot % KC2
                        pss = psum.tile([P, QT], F32, tag="mm", bufs=4,
                                        name="pss")
                        nc.tensor.matmul(
                            pss,
                            lhsT=_r(ksl[hf][:, ds(kc * P, P)]),
                            rhs=_r(qsl),
                        )
                        et = sb.tile([P, QT], F32, tag="exp", bufs=3,
                                     name="et")
                        nc.scalar.activation(_r(et), pss, AF.Exp, bias=nege)
                        if slot in mpos:
                            nc.vector.tensor_tensor(
                                _r(et), et, mt[:, mpos[slot]], op=OP.mult
                            )
                        nc.tensor.matmul(
                            psout,
                            lhsT=_r(vsl[hf][:, kc]),
                            rhs=_r(et),
                            start=(slot == 0),
                            stop=(slot == n_slots - 1),
                        )
                        nc.tensor.matmul(
                            psden,
                            lhsT=_r(ones_col),
                            rhs=_r(et),
                            start=(slot == 0),
                            stop=(slot == n_slots - 1),
                        )
                    rec0 = sb.tile([1, QT], F32, tag="stats_sb", bufs=4,
                                   name="rec0")
                    with nc.allow_low_precision(
                        reason="denominator reciprocal, 18 bits is plenty"
                    ):
                        nc.vector.reciprocal_approx_fast(rec0, psden)
                    rec = sb.tile([1, QT], F32, tag="stats_sb", bufs=4,
                                  name="rec")
                    nc.scalar.activation(_r(rec), rec0, AF.Copy)
                    psr = psum.tile([P, QT], F32, tag="stat", bufs=4,
                                    name="psr")
                    nc.tensor.matmul(psr, lhsT=_r(ones_row), rhs=_r(rec))
                    rsb = sb.tile([P, QT], F32, tag="tmp", bufs=3, name="rsb")
                    nc.scalar.activation(rsb, psr, AF.Copy)
                    ot = sb.tile([P, QT], F32, tag="outT", bufs=2, name="ot")
                    nc.vector.tensor_tensor(ot, psout, rsb, op=OP.mult)
                    nc.sync.dma_start(ots[:, h, ds(qsl_off, QT)], ot)

                # ---- Phase E: out-projection for this q tile ----
                EG = 4
                for eg in range(NH // EG):
                    psf = [
                        psum.tile([P, QT], F32, tag="mm", bufs=4, name="psf")
                        for _ in range(EG)
                    ]
                    for h in range(NH):
                        orh = sb.tile([P, QT], F32, tag="orhs", bufs=2,
                                      name="orh")
                        nc.sync.dma_start(_r(orh), _r(ots[:, h, ds(qsl_off, QT)]))
                        wo = sb.tile([P, EG * P], F32, tag="w", bufs=3,
                                     name="wo")
                        nc.sync.dma_start(
                            _r(wo), _r(woT[ds(h * P, P), ds(eg * EG * P, EG * P)])
                        )
                        for x in range(EG):
                            nc.tensor.matmul(
                                psf[x],
                                lhsT=_r(wo[:, ds(x * P, P)]),
                                rhs=_r(orh),
                                start=(h == 0),
                                stop=(h == NH - 1),
                            )
                    for x in range(EG):
                        fsb = sb.tile([P, QT], F32, tag="tmp", bufs=3,
                                      name="fsb")
                        nc.vector.tensor_copy(fsb, psf[x])
                        nc.sync.dma_start(
                            out_t[ds((eg * EG + x) * P, P), ds(qsl_off, QT)],
                            fsb,
                        )

    nc.compile()
    return nc


# --------------------------------------------------------------------------
# Host-side prep and driver
# --------------------------------------------------------------------------

def _q_blocks(role, n_blocks):
    """q-block indices (each 512 tokens) for a core role."""
    if n_blocks == 4:
        return [0, 3] if role == 0 else [1, 2]
    # degenerate small configs: one core covers all blocks
    return list(range(n_blocks))


def make_host_data(x, w_in, w_out, q_gamma, q_beta, k_gamma, k_beta, cfg,
                   n_cores=None):
    """Build per-core in_maps (list of dicts) + assembly metadata."""
    D = cfg["D"]
    S = cfg["S"]
    NQTOK = cfg["NQTOK"]
    QT = cfg["QT"]
    slots = cfg["slots"]
    masked = cfg["masked"]
    NH = D // HD
    NQ = NQTOK // QT
    MAXM = max(len(m) for m in masked)
    if cfg.get("MASK_F32", True):
        mdt = np.float32
    else:
        import ml_dtypes
        mdt = ml_dtypes.bfloat16
    B = x.shape[0]
    n_blocks = S // 512
    if n_cores is None:
        n_cores = B * (2048 // NQTOK) if S == 2048 else B

    w64 = np.asarray(w_in, np.float64)
    wq = w64[0:D]
    wk = w64[D:2 * D]
    wv = w64[2 * D:3 * D]
    wq_c = wq - wq.mean(axis=0, keepdims=True)
    wk_c = wk - wk.mean(axis=0, keepdims=True)
    wqkT2 = np.concatenate([wq_c.T, wk_c.T], axis=1).astype(np.float32)
    # pre-tile to [2*NH, P, DC, P]: tile ec -> [p, dc, e] with contiguous rows
    NHl = D // P
    DCl = D // P
    wqkT = np.ascontiguousarray(
        wqkT2.reshape(DCl, P, 2 * NHl, P).transpose(2, 1, 0, 3)
    )
    wvT = np.ascontiguousarray(wv.T.astype(np.float32))
    woT = np.ascontiguousarray(np.asarray(w_out, np.float64).T.astype(np.float32))

    inv = 1.0 / (10000.0 ** (np.arange(0, HD, 2, dtype=np.float64) / HD))
    tpos = np.arange(S, dtype=np.float64)
    fr = np.outer(tpos, inv)
    emb = np.concatenate([fr, fr], axis=-1)  # [S, HD]
    cosT = np.cos(emb).T  # [HD, S]
    sinTn = np.sin(emb).T

    # signed rotate-half permutation, as matmul lhsT:
    # out[p] = sum_{p'} rotmT[p', p] * in[p'] = rot_half(in)[p]
    h2 = HD // 2
    rotmT = np.zeros((P, P), np.float32)
    for p in range(h2):
        rotmT[p + h2, p] = -1.0
    for p in range(h2, HD):
        rotmT[p - h2, p] = 1.0

    scale = 1.0 / math.sqrt(HD)
    gq_a = np.ascontiguousarray(
        (np.asarray(q_gamma, np.float64) * scale).reshape(NH, P).T
    ).astype(np.float32)
    bq_a = np.ascontiguousarray(
        (np.asarray(q_beta, np.float64) * scale).reshape(NH, P).T
    ).astype(np.float32)
    gk_a = np.ascontiguousarray(
        np.asarray(k_gamma, np.float32).reshape(NH, P).T
    )
    bk_a = np.ascontiguousarray(
        np.asarray(k_beta, np.float32).reshape(NH, P).T
    )

    in_maps = []
    meta = []
    cores_per_batch = max(1, n_cores // B)
    for c in range(n_cores):
        b = c // cores_per_batch
        r = c % cores_per_batch
        blocks = _q_blocks(r if cores_per_batch > 1 else 0, n_blocks)
        blocks = blocks[: NQTOK // 512]
        qtok = np.concatenate(
            [np.arange(bk * 512, (bk + 1) * 512) for bk in blocks]
        )
        xb = np.asarray(x[b], np.float32)  # [S, D]
        xT = np.ascontiguousarray(xb.T)    # [D, S]
        xTq = np.ascontiguousarray(xT[:, qtok])
        cosq = np.ascontiguousarray(cosT[:, qtok].astype(np.float32))
        sinq = np.ascontiguousarray(sinTn[:, qtok].astype(np.float32))
        cosk = np.ascontiguousarray(cosT[:, :S].astype(np.float32))
        sink = np.ascontiguousarray(sinTn[:, :S].astype(np.float32))

        masks = np.zeros([NQ, P, MAXM, QT], np.float32)
        for t in range(NQ):
            q_start = blocks[t * (QT // 512)] * 512 if QT == 512 else None
            assert QT == 512
            q_start = blocks[t] * 512
            qq = np.arange(QT)
            kk = np.arange(P)
            for mi, kc in enumerate(masked[t]):
                masks[t, :, mi, :] = (
                    (kc * P + kk[:, None]) <= (q_start + qq[None, :])
                ).astype(np.float32)
        masks = masks.astype(mdt)

        in_maps.append(dict(
            xTq=xTq, xT=xT, wqkT=wqkT, wvT=wvT, woT=woT,
            cosq=cosq, sinqn=sinq, cosk=cosk, sinkn=sink,
            gq=gq_a, bq=bq_a, gk=gk_a, bk=bk_a, masks=masks,
            rotm=rotmT,
            onesc=np.ones((P, 1), np.float32),
            onesr=np.ones((1, P), np.float32),
        ))
        meta.append(dict(b=b, qtok=qtok))
    return in_maps, meta


_PROGRAM_CACHE = {}


def _get_program(cfg_key, cfg):
    if cfg_key not in _PROGRAM_CACHE:
        _PROGRAM_CACHE[cfg_key] = build_program(cfg)
    return _PROGRAM_CACHE[cfg_key]


def run_full(x, w_in, w_out, q_gamma, q_beta, k_gamma, k_beta,
             trace=False):
    from concourse.bass_utils import run_bass_kernel_spmd

    cfg = FULL_CFG
    B = x.shape[0]
    n_cores = 2 * B
    in_maps, meta = make_host_data(
        x, w_in, w_out, q_gamma, q_beta, k_gamma, k_beta, cfg,
        n_cores=n_cores,
    )
    nc = _get_program("full", cfg)
    res = run_bass_kernel_spmd(
        nc, in_maps, core_ids=list(range(n_cores)), trace=trace,
    )
    S, D = cfg["S"], cfg["D"]
    out = np.empty((B, S, D), np.float32)
    for c in range(n_cores):
        o = res.results[c]["out"]  # [D, NQTOK]
        out[meta[c]["b"], meta[c]["qtok"], :] = o.T
    return out, res


def kernel(x, w_in, w_out, q_gamma, q_beta, k_gamma, k_beta, n_heads=16,
           **_ignored):
    x = np.asarray(x, np.float32)
    assert int(np.asarray(n_heads)) * HD == x.shape[-1]
    out, _ = run_full(
        np.asarray(x, np.float32),
        np.asarray(w_in, np.float32),
        np.asarray(w_out, np.float32),
        np.asarray(q_gamma, np.float32),
        np.asarray(q_beta, np.float32),
        np.asarray(k_gamma, np.float32),
        np.asarray(k_beta, np.float32),
    )
    return out



# revision 13
# speedup vs baseline: 1.9141x; 1.9141x over previous
"""Trainium2 Bass kernel for a custom attention block (qkv-proj + LN(q,k) +
RoPE + causal attention + out-proj), distributed over 8 NeuronCores.

Sharding: 2 cores per batch (B=4). Core role r=c%2 takes q-token blocks
{0,3} (r=0) or {1,2} (r=1) of 512 tokens; every core computes K/V for the
full 2048-token sequence of its batch (no collectives). The compiled
program is identical on all cores; all per-core differences are input
data (sliced x^T, cos/sin tables, causal masks).

v2 (bf16): all matmul operands are bf16 (fp32 PSUM accumulation), which
enables fast weight load and 1024-wide moving operands and halves DMA
traffic vs fp32. K stays SBUF-resident from projection through attention
(8 MB) so attention never re-reads K from HBM; V and q round-trip
through DRAM scratch with deep prefetch. Weights stream from HBM once
per 1024-token x pair. Scores for two kv-chunks share one PSUM tile so
exp runs on [128,1024] activations (ACT at ~1 elem/cycle/lane is the
attention co-bottleneck). LN variance uses Square + ones-matmul
partition reduction with rsqrt computed as Exp(-0.5*Ln(var+eps));
softmax is exp(s-8) with data-driven causal mask multiplies and a
ones-matmul denominator. LN-apply/rope DVE work for each phase is
emitted interleaved into the NEXT phase's matmul stream so the in-order
PE never head-of-line blocks on DVE.
"""

import math

import numpy as np

import concourse.bass as bass
import concourse.mybir as mybir
import concourse.tile as tile
from concourse import bacc
from concourse.bass import ds, ts

F32 = mybir.dt.float32
BF16 = mybir.dt.bfloat16
AF = mybir.ActivationFunctionType
OP = mybir.AluOpType

P = 128
HD = 128

FULL_CFG = dict(
    D=2048,           # model dim (contraction dim for projections)
    S=2048,           # kv tokens per core (full sequence of its batch)
    NQTOK=1024,       # q tokens per core
    QT=512,           # attention q-tile width (moving dim)
    slots=(8, 16),    # kv 128-chunks visited per q-tile
    masked=(tuple(range(0, 8)), tuple(range(8, 16))),  # slots that get a mask
    EXP_BIAS=8.0,
    EPS=1e-5,
)


def build_program(cfg):
    D = cfg["D"]
    S = cfg["S"]
    NQTOK = cfg["NQTOK"]
    QT = cfg["QT"]
    slots = cfg["slots"]
    masked = cfg["masked"]
    EXP_BIAS = cfg["EXP_BIAS"]
    EPS = cfg["EPS"]

    NH = D // HD              # heads == e-chunks per q (and per k)
    DC = D // P               # contraction chunks
    NQ = NQTOK // QT          # q tiles
    KC = S // P               # kv chunks
    MAXM = max(len(m) for m in masked)

    nc = bacc.Bacc("TRN2", target_bir_lowering=False, debug=False)

    # ---- I/O ----
    xT = nc.dram_tensor("xT", [D, S], BF16, kind="ExternalInput").ap()
    xTq = nc.dram_tensor("xTq", [D, NQTOK], BF16, kind="ExternalInput").ap()
    wqkT = nc.dram_tensor("wqkT", [2 * NH, P, DC, P], BF16,
                          kind="ExternalInput").ap()
    wvTt = nc.dram_tensor("wvTt", [NH, P, DC, P], BF16,
                          kind="ExternalInput").ap()
    ident_i = nc.dram_tensor("ident", [P, P], BF16,
                             kind="ExternalInput").ap()
    woTt = nc.dram_tensor("woTt", [DC, P, NH, P], BF16,
                          kind="ExternalInput").ap()
    cosq_i = nc.dram_tensor("cosq", [HD, NQTOK], BF16,
                            kind="ExternalInput").ap()
    sinq_i = nc.dram_tensor("sinqn", [HD, NQTOK], BF16,
                            kind="ExternalInput").ap()
    cosk_i = nc.dram_tensor("cosk", [HD, S], BF16, kind="ExternalInput").ap()
    sink_i = nc.dram_tensor("sinkn", [HD, S], BF16, kind="ExternalInput").ap()
    rotm_i = nc.dram_tensor("rotm", [P, P], BF16, kind="ExternalInput").ap()
    onesc_i = nc.dram_tensor("onesc", [P, 1], BF16, kind="ExternalInput").ap()
    onesr_i = nc.dram_tensor("onesr", [1, P], BF16, kind="ExternalInput").ap()
    gq_i = nc.dram_tensor("gq", [P, NH], F32, kind="ExternalInput").ap()
    bq_i = nc.dram_tensor("bq", [P, NH], F32, kind="ExternalInput").ap()
    gk_i = nc.dram_tensor("gk", [P, NH], F32, kind="ExternalInput").ap()
    bk_i = nc.dram_tensor("bk", [P, NH], F32, kind="ExternalInput").ap()
    masks_i = nc.dram_tensor("masks", [NQ, P, MAXM, QT], BF16,
                             kind="ExternalInput").ap()
    out_t = nc.dram_tensor("out", [D, NQTOK], F32, kind="ExternalOutput").ap()

    with tile.TileContext(nc) as tc:
        import contextlib

        ctx = contextlib.ExitStack()
        with ctx:
            sb = ctx.enter_context(tc.tile_pool(name="sb", bufs=1))
            psum = ctx.enter_context(tc.tile_pool(name="ps", bufs=1,
                                                  space="PSUM"))
            dram = ctx.enter_context(tc.tile_pool(name="dram", bufs=1,
                                                  space="DRAM"))

            # ---- DRAM scratch ----
            qts = dram.tile([NQ, NH, P, QT], BF16, tag="qts", name="qts")
            vts = dram.tile([NH, S, HD], BF16, tag="vts", name="vts")

            # ---- persistent SBUF: K slab + small constants ----
            kslab = sb.tile([P, NH, S], BF16, tag="kslab", name="kslab")

            ones_col = sb.tile([P, 1], BF16, tag="ones_col", name="ones_col")
            nc.sync.dma_start(ones_col, onesc_i)
            ones_row = sb.tile([1, P], BF16, tag="ones_row", name="ones_row")
            nc.sync.dma_start(ones_row, onesr_i)
            eps1 = sb.tile([1, 1], F32, tag="eps1", name="eps1")
            nc.vector.memset(eps1, EPS)
            zero1 = sb.tile([1, 1], F32, tag="zero1", name="zero1")
            nc.vector.memset(zero1, 0.0)
            nege = sb.tile([P, 1], F32, tag="nege", name="nege")
            nc.vector.memset(nege, -EXP_BIAS)
            rotm = sb.tile([P, P], BF16, tag="rotm", name="rotm")
            nc.sync.dma_start(rotm, rotm_i)
            ident = sb.tile([P, P], BF16, tag="ident", name="ident")
            nc.sync.dma_start(ident, ident_i)
            gq = sb.tile([P, NH], F32, tag="gq", name="gq")
            nc.sync.dma_start(gq, gq_i)
            bq = sb.tile([P, NH], F32, tag="bq", name="bq")
            nc.sync.dma_start(bq, bq_i)
            gk = sb.tile([P, NH], F32, tag="gk", name="gk")
            nc.sync.dma_start(gk, gk_i)
            bk = sb.tile([P, NH], F32, tag="bk", name="bk")
            nc.sync.dma_start(bk, bk_i)

            proj_ctx = contextlib.ExitStack()
            proj = proj_ctx.enter_context(tc.tile_pool(name="proj", bufs=1))

            def stats_finish(pstat):
                """pstat [1, QT] sumsq psum -> [P, QT] bf16 rsig broadcast."""
                lnv = proj.tile([1, QT], F32, tag="stats_sb", bufs=4,
                                name="lnv")
                nc.scalar.activation(lnv, pstat, AF.Ln, scale=1.0 / D,
                                     bias=eps1)
                rsig = proj.tile([1, QT], BF16, tag="stats_sb", bufs=4,
                                 name="rsig")
                nc.scalar.activation(rsig, lnv, AF.Exp, bias=zero1,
                                     scale=-0.5)
                ps_rep = psum.tile([P, QT], F32, tag="sc", bufs=2,
                                   name="ps_rep")
                nc.tensor.matmul(ps_rep, lhsT=ones_row, rhs=rsig)
                rsb = proj.tile([P, QT], BF16, tag="rsb", bufs=4, name="rsb")
                nc.vector.tensor_copy(rsb, ps_rep)
                return rsb

            def ln_apply(sl, rsb, g_sb, b_sb, ec):
                """In-place LN affine on a [P, QT] bf16 slab slice."""
                nc.vector.tensor_tensor(sl, sl, rsb, op=OP.mult)
                nc.vector.tensor_scalar(
                    sl, sl,
                    scalar1=g_sb[:, ds(ec, 1)],
                    scalar2=b_sb[:, ds(ec, 1)],
                    op0=OP.mult, op1=OP.add,
                )

            def rope_apply(sl, cos_sl, sin_sl):
                """In-place rope on a [P, QT] bf16 slab slice."""
                ps_rot = psum.tile([P, QT], F32, tag="sc", bufs=2,
                                   name="ps_rot")
                nc.tensor.matmul(ps_rot, lhsT=rotm, rhs=sl)
                tmp = proj.tile([P, QT], BF16, tag="tmp", bufs=2, name="tmp")
                nc.vector.tensor_tensor(tmp, ps_rot, sin_sl, op=OP.mult)
                nc.vector.tensor_tensor(sl, sl, cos_sl, op=OP.mult)
                nc.vector.tensor_tensor(sl, sl, tmp, op=OP.add)

            # ---------------- emission helpers -----------------------
            # Deferred-work queues: each entry is a closure emitting the
            # LN+rope (DVE + 1 small PE matmul) for one head chunk. They
            # are drained interleaved into the next phase's matmul stream
            # so the PE never stalls on DVE and the DVE FIFO never blocks
            # a PSUM-slot reuse.
            def kproj_pair(pr, drain):
                """k projection for 1024-token pair pr; returns (rsbs,
                deferred ln+rope closures). drain: list of closures to
                interleave (one per ec)."""
                tok0 = pr * 1024
                xb = proj.tile([P, DC, 1024], BF16, tag="xb", bufs=1,
                               name="xb")
                for d in range(DC):
                    nc.sync.dma_start(xb[:, d],
                                      xT[ds(d * P, P), ds(tok0, 1024)])
                pstats = [
                    psum.tile([1, QT], F32, tag="st", bufs=2, name="pstat_k")
                    for _ in range(2)
                ]
                for ec in range(NH):
                    w = proj.tile([P, DC, P], BF16, tag="w", bufs=3,
                                  name="wk")
                    nc.sync.dma_start(w, wqkT[NH + ec])
                    psk = psum.tile([P, 1024], F32, tag="pair", bufs=2,
                                    name="psk")
                    for d in range(DC):
                        for half in range(2):
                            nc.tensor.matmul(
                                psk[:, ds(half * QT, QT)], lhsT=w[:, d],
                                rhs=xb[:, d, ds(half * QT, QT)],
                                start=(d == 0), stop=(d == DC - 1),
                            )
                    nc.vector.tensor_copy(kslab[:, ec, ds(tok0, 1024)], psk)
                    sq = proj.tile([P, 1024], BF16, tag="sq", bufs=2,
                                   name="sqk")
                    nc.scalar.square(sq, psk)
                    for half in range(2):
                        nc.tensor.matmul(pstats[half], lhsT=ones_col,
                                         rhs=sq[:, ds(half * QT, QT)],
                                         start=(ec == 0), stop=(ec == NH - 1))
                    if ec < len(drain):
                        drain[ec]()
                rsbs = [stats_finish(pstats[half]) for half in range(2)]
                return xb, rsbs

            def k_lnrope_closures(pr, rsbs, cosk, sink):
                tok0 = pr * 1024
                out = []
                for ec in range(NH):
                    def fn(ec=ec):
                        for half in range(2):
                            sl = kslab[:, ec, ds(tok0 + half * QT, QT)]
                            ln_apply(sl, rsbs[half], gk, bk, ec)
                            rope_apply(sl, cosk[:, ds(half * QT, QT)],
                                       sink[:, ds(half * QT, QT)])
                    out.append(fn)
                return out

            def vproj_pair(pr, xb, drain):
                """v projection for pair pr using resident xb; writes vts
                (token-major) via per-head feature-major matmuls + PE block
                transposes. drain: closures interleaved per head."""
                for ecv in range(NH):
                    wvc = proj.tile([P, DC, P], BF16, tag="w", bufs=3,
                                    name="wvc")
                    nc.sync.dma_start(wvc, wvTt[ecv])
                    psvf = psum.tile([P, 1024], F32, tag="pair", bufs=2,
                                     name="psvf")
                    for d in range(DC):
                        for half in range(2):
                            nc.tensor.matmul(
                                psvf[:, ds(half * QT, QT)], lhsT=wvc[:, d],
                                rhs=xb[:, d, ds(half * QT, QT)],
                                start=(d == 0), stop=(d == DC - 1),
                            )
                    vfsb = proj.tile([P, 1024], BF16, tag="vsb", bufs=3,
                                     name="vfsb")
                    nc.vector.tensor_copy(vfsb, psvf)
                    vtsb = proj.tile([P, 8, HD], BF16, tag="vtsb", bufs=2,
                                     name="vtsb")
                    for tc8 in range(8):
                        pst = psum.tile([P, P], BF16, tag="st", bufs=2,
                                        name="pst")
                        nc.tensor.transpose(pst, vfsb[:, ds(tc8 * P, P)],
                                            ident)
                        nc.vector.tensor_copy(vtsb[:, tc8], pst)
                    nc.sync.dma_start(
                        vts[ecv, ds(pr * 1024, 1024), :].rearrange(
                            "(kc p) hd -> p kc hd", p=P
                        ),
                        vtsb,
                    )
                    if ecv < len(drain):
                        drain[ecv]()

            def qproj_tile(t, drain):
                """q projection for 512-token tile t into holdq; stats
                finished; returns (holdq, rsb)."""
                xq = proj.tile([P, DC, QT], BF16, tag="xq", bufs=1,
                               name="xq")
                for d in range(DC):
                    nc.sync.dma_start(xq[:, d],
                                      xTq[ds(d * P, P), ds(t * QT, QT)])
                holdq = proj.tile([P, NH, QT], BF16, tag="holdq", bufs=2,
                                  name="holdq")
                pstat = psum.tile([1, QT], F32, tag="st", bufs=2,
                                  name="pstat_q")
                for ec in range(NH):
                    w = proj.tile([P, DC, P], BF16, tag="w", bufs=3,
                                  name="wq")
                    nc.sync.dma_start(w, wqkT[ec])
                    psq = psum.tile([P, QT], F32, tag="sc", bufs=2,
                                    name="psq")
                    for d in range(DC):
                        nc.tensor.matmul(psq, lhsT=w[:, d], rhs=xq[:, d],
                                         start=(d == 0), stop=(d == DC - 1))
                    nc.vector.tensor_copy(holdq[:, ec], psq)
                    sq = proj.tile([P, QT], BF16, tag="sq", bufs=2,
                                   name="sqq")
                    nc.scalar.square(sq, psq)
                    nc.tensor.matmul(pstat, lhsT=ones_col, rhs=sq,
                                     start=(ec == 0), stop=(ec == NH - 1))
                    if ec < len(drain):
                        drain[ec]()
                rsb = stats_finish(pstat)
                return holdq, rsb

            def q_lnrope_closures(t, holdq, rsb, cosq, sinq):
                out = []
                for ec in range(NH):
                    def fn(ec=ec, last=(ec == NH - 1)):
                        sl = holdq[:, ec]
                        ln_apply(sl, rsb, gq, bq, ec)
                        rope_apply(sl, cosq, sinq)
                        if last:
                            nc.sync.dma_start(
                                qts[t].rearrange("h p q -> p h q"), holdq
                            )
                    out.append(fn)
                return out

            # ---------------- projection schedule ---------------------
            cosks = {}
            sinks = {}
            for pr in range(2):
                cosks[pr] = proj.tile([HD, 1024], BF16, tag="cs2", bufs=2,
                                      name="cosk")
                nc.sync.dma_start(cosks[pr], cosk_i[:, ds(pr * 1024, 1024)])
                sinks[pr] = proj.tile([HD, 1024], BF16, tag="cs2", bufs=2,
                                      name="sink")
                nc.sync.dma_start(sinks[pr], sink_i[:, ds(pr * 1024, 1024)])
            cosqs = {}
            sinqs = {}
            for t in range(NQ):
                cosqs[t] = proj.tile([HD, QT], BF16, tag="cs", bufs=4,
                                     name="cosq")
                nc.sync.dma_start(cosqs[t], cosq_i[:, ds(t * QT, QT)])
                sinqs[t] = proj.tile([HD, QT], BF16, tag="cs", bufs=4,
                                     name="sinq")
                nc.sync.dma_start(sinqs[t], sinq_i[:, ds(t * QT, QT)])

            # pair 0: k + v, then its LN/rope drains into q-proj t0
            xb0, rsbs0 = kproj_pair(0, drain=[])
            vproj_pair(0, xb0, drain=[])
            ln0 = k_lnrope_closures(0, rsbs0, cosks[0], sinks[0])

            hq0, rsbq0 = qproj_tile(0, drain=ln0)
            lnq0 = q_lnrope_closures(0, hq0, rsbq0, cosqs[0], sinqs[0])
            hq1, rsbq1 = qproj_tile(1, drain=lnq0)
            lnq1 = q_lnrope_closures(1, hq1, rsbq1, cosqs[1], sinqs[1])

            # pair 1: k (draining q1's LN/rope), then v (draining k1's)
            xb1, rsbs1 = kproj_pair(1, drain=lnq1)
            ln1 = k_lnrope_closures(1, rsbs1, cosks[1], sinks[1])
            vproj_pair(1, xb1, drain=ln1)

            proj_ctx.close()

            # ============ attention + out-projection ===========
            attn_ctx = contextlib.ExitStack()
            attn = attn_ctx.enter_context(tc.tile_pool(name="attn", bufs=1))
            ot = attn.tile([P, NH, NQTOK], BF16, tag="ot", name="ot")
            PRE = 2  # score pair lookahead

            for t in range(NQ):
                mt = attn.tile([P, MAXM, QT], BF16, tag="masks", bufs=2,
                               name="mt")
                nc.sync.dma_start(mt, masks_i[t])
                mpos = {kc: i for i, kc in enumerate(masked[t])}
                n_slots = slots[t]
                n_pairs = n_slots // 2
                for h in range(NH):
                    qsl = attn.tile([P, QT], BF16, tag="qslab", bufs=3,
                                    name="qsl")
                    nc.sync.dma_start(qsl, qts[t, h])
                    vsl = attn.tile([P, KC, HD], BF16, tag="vslab", bufs=3,
                                    name="vsl")
                    nc.sync.dma_start(
                        vsl[:, :n_slots],
                        vts[h, ds(0, n_slots * P), :].rearrange(
                            "(kc p) hd -> p kc hd", p=P
                        ),
                    )
                    psout = psum.tile([P, QT], F32, tag="sc", bufs=2,
                                      name="psout")
                    psden = psum.tile([1, QT], F32, tag="st", bufs=2,
                                      name="psden")

                    ps_pairs = {}

                    def issue_scores(p, h=h, qsl=qsl, ps_pairs=ps_pairs):
                        pp = psum.tile([P, 1024], F32, tag="pair", bufs=2,
                                       name="pp")
                        for half in range(2):
                            kc = 2 * p + half
                            nc.tensor.matmul(
                                pp[:, ds(half * QT, QT)],
                                lhsT=kslab[:, h, ds(kc * P, P)],
                                rhs=qsl,
                            )
                        ps_pairs[p] = pp

                    for p in range(min(PRE, n_pairs)):
                        issue_scores(p)
                    for p in range(n_pairs):
                        if p + PRE < n_pairs:
                            issue_scores(p + PRE)
                        pp = ps_pairs.pop(p)
                        et2 = attn.tile([P, 1024], BF16, tag="exp", bufs=3,
                                        name="et2")
                        nc.scalar.activation(et2, pp, AF.Exp, bias=nege)
                        for half in range(2):
                            kc = 2 * p + half
                            if kc in mpos:
                                nc.vector.tensor_tensor(
                                    et2[:, ds(half * QT, QT)],
                                    et2[:, ds(half * QT, QT)],
                                    mt[:, mpos[kc]], op=OP.mult,
                                )
                        for half in range(2):
                            kc = 2 * p + half
                            nc.tensor.matmul(
                                psout,
                                lhsT=vsl[:, kc],
                                rhs=et2[:, ds(half * QT, QT)],
                                start=(kc == 0), stop=(kc == n_slots - 1),
                            )
                        for half in range(2):
                            kc = 2 * p + half
                            nc.tensor.matmul(
                                psden, lhsT=ones_col,
                                rhs=et2[:, ds(half * QT, QT)],
                                start=(kc == 0), stop=(kc == n_slots - 1),
                            )

                    rec0 = attn.tile([1, QT], F32, tag="stats_sb", bufs=4,
                                     name="rec0")
                    with nc.allow_low_precision(
                        reason="denominator reciprocal, 18 bits is plenty"
                    ):
                        nc.vector.reciprocal_approx_fast(rec0, psden)
                    rec = attn.tile([1, QT], BF16, tag="stats_sb", bufs=4,
                                    name="rec")
                    nc.scalar.activation(rec, rec0, AF.Copy)
                    psr = psum.tile([P, QT], F32, tag="sc", bufs=2,
                                    name="psr")
                    nc.tensor.matmul(psr, lhsT=ones_row, rhs=rec)
                    rsb = attn.tile([P, QT], BF16, tag="rsbn", bufs=2,
                                    name="rsbn")
                    nc.vector.tensor_copy(rsb, psr)
                    nc.vector.tensor_tensor(ot[:, h, ds(t * QT, QT)], psout,
                                            rsb, op=OP.mult)

            # ---- out-projection over all q tokens ----
            for ec in range(DC):
                woc = attn.tile([P, NH, P], BF16, tag="woc", bufs=3,
                                name="woc")
                nc.sync.dma_start(woc, woTt[ec])
                psf = psum.tile([P, 1024], F32, tag="pair", bufs=2,
                                name="psf")
                for h in range(NH):
                    for half in range(2):
                        nc.tensor.matmul(
                            psf[:, ds(half * QT, QT)], lhsT=woc[:, h],
                            rhs=ot[:, h, ds(half * QT, QT)],
                            start=(h == 0), stop=(h == NH - 1),
                        )
                fsb = attn.tile([P, 1024], F32, tag="fsb", bufs=2, name="fsb")
                nc.vector.tensor_copy(fsb, psf)
                nc.sync.dma_start(out_t[ds(ec * P, P), :], fsb)

            attn_ctx.close()

    nc.compile()
    return nc


# --------------------------------------------------------------------------
# Host-side prep and driver
# --------------------------------------------------------------------------

def _q_blocks(role):
    """q-block indices (each 512 tokens) for a core role."""
    return [0, 3] if role == 0 else [1, 2]


def make_host_data(x, w_in, w_out, q_gamma, q_beta, k_gamma, k_beta, cfg,
                   n_cores=None):
    """Build per-core in_maps (list of dicts) + assembly metadata."""
    import ml_dtypes

    BF = ml_dtypes.bfloat16
    D = cfg["D"]
    S = cfg["S"]
    NQTOK = cfg["NQTOK"]
    QT = cfg["QT"]
    masked = cfg["masked"]
    NH = D // HD
    DC = D // P
    NQ = NQTOK // QT
    MAXM = max(len(m) for m in masked)
    B = x.shape[0]
    if n_cores is None:
        n_cores = 2 * B

    w64 = np.asarray(w_in, np.float64)
    wq = w64[0:D]
    wk = w64[D:2 * D]
    wv = w64[2 * D:3 * D]
    wq_c = wq - wq.mean(axis=0, keepdims=True)
    wk_c = wk - wk.mean(axis=0, keepdims=True)
    wqkT2 = np.concatenate([wq_c.T, wk_c.T], axis=1).astype(BF)
    # pre-tile to [2*NH, P, DC, P]: tile ec -> [p, dc, e] with contiguous rows
    wqkT = np.ascontiguousarray(
        wqkT2.reshape(DC, P, 2 * NH, P).transpose(2, 1, 0, 3)
    )
    wvTt = np.ascontiguousarray(
        wv.T.astype(BF).reshape(DC, P, NH, P).transpose(2, 1, 0, 3)
    )
    woT = np.asarray(w_out, np.float64).T  # [feat, e]
    # [ec, p(feat within head), h, e-col]
    woTt = np.ascontiguousarray(
        woT.reshape(NH, P, DC, P).transpose(2, 1, 0, 3).astype(BF)
    )

    inv = 1.0 / (10000.0 ** (np.arange(0, HD, 2, dtype=np.float64) / HD))
    tpos = np.arange(S, dtype=np.float64)
    fr = np.outer(tpos, inv)
    emb = np.concatenate([fr, fr], axis=-1)  # [S, HD]
    cosT = np.cos(emb).T  # [HD, S]
    sinTn = np.sin(emb).T

    # signed rotate-half permutation, as matmul lhsT:
    # out[p] = sum_{p'} rotmT[p', p] * in[p'] = rot_half(in)[p]
    h2 = HD // 2
    rotmT = np.zeros((P, P), np.float32)
    for p in range(h2):
        rotmT[p + h2, p] = -1.0
    for p in range(h2, HD):
        rotmT[p - h2, p] = 1.0

    scale = 1.0 / math.sqrt(HD)
    gq_a = np.ascontiguousarray(
        (np.asarray(q_gamma, np.float64) * scale).reshape(NH, P).T
    ).astype(np.float32)
    bq_a = np.ascontiguousarray(
        (np.asarray(q_beta, np.float64) * scale).reshape(NH, P).T
    ).astype(np.float32)
    gk_a = np.ascontiguousarray(
        np.asarray(k_gamma, np.float32).reshape(NH, P).T
    )
    bk_a = np.ascontiguousarray(
        np.asarray(k_beta, np.float32).reshape(NH, P).T
    )

    cosk = np.ascontiguousarray(cosT.astype(BF))
    sink = np.ascontiguousarray(sinTn.astype(BF))

    in_maps = []
    meta = []
    cores_per_batch = max(1, n_cores // B)
    for c in range(n_cores):
        b = c // cores_per_batch
        r = c % cores_per_batch
        blocks = _q_blocks(r)
        qtok = np.concatenate(
            [np.arange(bk * 512, (bk + 1) * 512) for bk in blocks]
        )
        xb = np.asarray(x[b], np.float32)  # [S, D]
        xTf = xb.T.astype(BF)              # [D, S]
        xT = np.ascontiguousarray(xTf)
        xTq = np.ascontiguousarray(xTf[:, qtok])
        cosq = np.ascontiguousarray(cosT[:, qtok].astype(BF))
        sinq = np.ascontiguousarray(sinTn[:, qtok].astype(BF))

        masks = np.zeros([NQ, P, MAXM, QT], np.float32)
        for t in range(NQ):
            q_start = blocks[t] * 512
            qq = np.arange(QT)
            kk = np.arange(P)
            for mi, kc in enumerate(masked[t]):
                masks[t, :, mi, :] = (
                    (kc * P + kk[:, None]) <= (q_start + qq[None, :])
                ).astype(np.float32)
        masks = masks.astype(BF)

        in_maps.append(dict(
            xTq=xTq, xT=xT, wqkT=wqkT, wvTt=wvTt, woTt=woTt,
            cosq=cosq, sinqn=sinq, cosk=cosk, sinkn=sink,
            gq=gq_a, bq=bq_a, gk=gk_a, bk=bk_a, masks=masks,
            rotm=rotmT.astype(BF),
            ident=np.eye(P, dtype=np.float32).astype(BF),
            onesc=np.ones((P, 1), BF),
            onesr=np.ones((1, P), BF),
        ))
        meta.append(dict(b=b, qtok=qtok))
    return in_maps, meta


_PROGRAM_CACHE = {}


def _get_program(cfg_key, cfg):
    if cfg_key not in _PROGRAM_CACHE:
        _PROGRAM_CACHE[cfg_key] = build_program(cfg)
    return _PROGRAM_CACHE[cfg_key]


def run_full(x, w_in, w_out, q_gamma, q_beta, k_gamma, k_beta,
             trace=False):
    from concourse.bass_utils import run_bass_kernel_spmd

    cfg = FULL_CFG
    B = x.shape[0]
    n_cores = 2 * B
    in_maps, meta = make_host_data(
        x, w_in, w_out, q_gamma, q_beta, k_gamma, k_beta, cfg,
        n_cores=n_cores,
    )
    nc = _get_program("full", cfg)
    res = run_bass_kernel_spmd(
        nc, in_maps, core_ids=list(range(n_cores)), trace=trace,
    )
    S, D = cfg["S"], cfg["D"]
    out = np.empty((B, S, D), np.float32)
    for c in range(n_cores):
        o = res.results[c]["out"]  # [D, NQTOK]
        out[meta[c]["b"], meta[c]["qtok"], :] = o.T
    return out, res


def kernel(x, w_in, w_out, q_gamma, q_beta, k_gamma, k_beta, n_heads=16,
           **_ignored):
    x = np.asarray(x, np.float32)
    assert int(np.asarray(n_heads)) * HD == x.shape[-1]
    out, _ = run_full(
        np.asarray(x, np.float32),
        np.asarray(w_in, np.float32),
        np.asarray(w_out, np.float32),
        np.asarray(q_gamma, np.float32),
        np.asarray(q_beta, np.float32),
        np.asarray(k_gamma, np.float32),
        np.asarray(k_beta, np.float32),
    )
    return out


# revision 22
# speedup vs baseline: 2.0658x; 1.0793x over previous
"""Trainium2 Bass kernel for a custom attention block (qkv-proj + LN(q,k) +
RoPE + causal attention + out-proj), distributed over 8 NeuronCores.

Sharding: 2 cores per batch (B=4). Core role r=c%2 takes q-token blocks
{0,3} (r=0) or {1,2} (r=1) of 512 tokens; every core computes K/V for the
full 2048-token sequence of its batch (no collectives). The compiled
program is identical on all cores; all per-core differences are input
data (sliced x^T, cos/sin tables, causal masks).

v2 (bf16): all matmul operands are bf16 (fp32 PSUM accumulation), which
enables fast weight load and 1024-wide moving operands and halves DMA
traffic vs fp32. K stays SBUF-resident from projection through attention
(8 MB) so attention never re-reads K from HBM; V and q round-trip
through DRAM scratch with deep prefetch. Weights stream from HBM once
per 1024-token x pair. Scores for two kv-chunks share one PSUM tile so
exp runs on [128,1024] activations (ACT at ~1 elem/cycle/lane is the
attention co-bottleneck). LN variance uses Square + ones-matmul
partition reduction with rsqrt computed as Exp(-0.5*Ln(var+eps));
softmax is exp(s-8) with data-driven causal mask multiplies and a
ones-matmul denominator. LN-apply/rope DVE work for each phase is
emitted interleaved into the NEXT phase's matmul stream so the in-order
PE never head-of-line blocks on DVE.
"""

import math

import numpy as np

import concourse.bass as bass
import concourse.mybir as mybir
import concourse.tile as tile
from concourse import bacc
from concourse.bass import ds, ts

F32 = mybir.dt.float32
BF16 = mybir.dt.bfloat16
AF = mybir.ActivationFunctionType
OP = mybir.AluOpType

P = 128
HD = 128

FULL_CFG = dict(
    D=2048,           # model dim (contraction dim for projections)
    S=2048,           # kv tokens per core (full sequence of its batch)
    NQTOK=1024,       # q tokens per core
    QT=512,           # attention q-tile width (moving dim)
    slots=(8, 16),    # kv 128-chunks visited per q-tile
    masked=(tuple(range(0, 8)), tuple(range(8, 16))),  # slots that get a mask
    EXP_BIAS=8.0,
    EPS=1e-5,
)


def build_program(cfg):
    D = cfg["D"]
    S = cfg["S"]
    NQTOK = cfg["NQTOK"]
    QT = cfg["QT"]
    slots = cfg["slots"]
    masked = cfg["masked"]
    EXP_BIAS = cfg["EXP_BIAS"]
    EPS = cfg["EPS"]

    NH = D // HD              # heads == e-chunks per q (and per k)
    DC = D // P               # contraction chunks
    NQ = NQTOK // QT          # q tiles
    KC = S // P               # kv chunks
    MAXM = max(len(m) for m in masked)

    nc = bacc.Bacc("TRN2", target_bir_lowering=False, debug=False)

    # ---- I/O ----
    xT = nc.dram_tensor("xT", [D, S], BF16, kind="ExternalInput").ap()
    xTq = nc.dram_tensor("xTq", [D, NQTOK], BF16, kind="ExternalInput").ap()
    wqkT = nc.dram_tensor("wqkT", [2 * NH, P, DC, P], BF16,
                          kind="ExternalInput").ap()
    wvTt = nc.dram_tensor("wvTt", [NH, P, DC, P], BF16,
                          kind="ExternalInput").ap()
    ident_i = nc.dram_tensor("ident", [P, P], BF16,
                             kind="ExternalInput").ap()
    woTt = nc.dram_tensor("woTt", [DC, P, NH, P], BF16,
                          kind="ExternalInput").ap()
    cosq_i = nc.dram_tensor("cosq", [HD, NQTOK], BF16,
                            kind="ExternalInput").ap()
    sinq_i = nc.dram_tensor("sinqn", [HD, NQTOK], BF16,
                            kind="ExternalInput").ap()
    cosk_i = nc.dram_tensor("cosk", [HD, S], BF16, kind="ExternalInput").ap()
    sink_i = nc.dram_tensor("sinkn", [HD, S], BF16, kind="ExternalInput").ap()
    rotm_i = nc.dram_tensor("rotm", [P, P], BF16, kind="ExternalInput").ap()
    onesc_i = nc.dram_tensor("onesc", [P, 1], BF16, kind="ExternalInput").ap()
    onesr_i = nc.dram_tensor("onesr", [1, P], BF16, kind="ExternalInput").ap()
    gq_i = nc.dram_tensor("gq", [P, NH], F32, kind="ExternalInput").ap()
    bq_i = nc.dram_tensor("bq", [P, NH], F32, kind="ExternalInput").ap()
    gk_i = nc.dram_tensor("gk", [P, NH], F32, kind="ExternalInput").ap()
    bk_i = nc.dram_tensor("bk", [P, NH], F32, kind="ExternalInput").ap()
    masks_i = nc.dram_tensor("masks", [NQ, P, MAXM, QT], BF16,
                             kind="ExternalInput").ap()
    out_t = nc.dram_tensor("out", [D, NQTOK], F32, kind="ExternalOutput").ap()

    with tile.TileContext(nc) as tc:
        import contextlib

        ctx = contextlib.ExitStack()
        with ctx:
            sb = ctx.enter_context(tc.tile_pool(name="sb", bufs=1))
            psum = ctx.enter_context(tc.tile_pool(name="ps", bufs=1,
                                                  space="PSUM"))
            dram = ctx.enter_context(tc.tile_pool(name="dram", bufs=1,
                                                  space="DRAM"))

            # ---- DRAM scratch ----
            qts = dram.tile([NQ, NH, P, QT], BF16, tag="qts", name="qts")
            vts = dram.tile([NH, S, HD], BF16, tag="vts", name="vts")

            # ---- persistent SBUF: K slabs (one per 1024-token pair, so
            # attention tile 0 never false-depends on pair-1 writes) ----
            kslabs = [
                sb.tile([P, NH, 1024], BF16, tag=f"kslab{pr}",
                        name=f"kslab{pr}")
                for pr in range(2)
            ]

            ones_col = sb.tile([P, 1], BF16, tag="ones_col", name="ones_col")
            nc.sync.dma_start(ones_col, onesc_i)
            ones_row = sb.tile([1, P], BF16, tag="ones_row", name="ones_row")
            nc.sync.dma_start(ones_row, onesr_i)
            eps1 = sb.tile([1, 1], F32, tag="eps1", name="eps1")
            nc.vector.memset(eps1, EPS)
            zero1 = sb.tile([1, 1], F32, tag="zero1", name="zero1")
            nc.vector.memset(zero1, 0.0)
            nege = sb.tile([P, 1], F32, tag="nege", name="nege")
            nc.vector.memset(nege, -EXP_BIAS)
            rotm = sb.tile([P, P], BF16, tag="rotm", name="rotm")
            nc.sync.dma_start(rotm, rotm_i)
            ident = sb.tile([P, P], BF16, tag="ident", name="ident")
            nc.sync.dma_start(ident, ident_i)
            gq = sb.tile([P, NH], F32, tag="gq", name="gq")
            nc.sync.dma_start(gq, gq_i)
            bq = sb.tile([P, NH], F32, tag="bq", name="bq")
            nc.sync.dma_start(bq, bq_i)
            gk = sb.tile([P, NH], F32, tag="gk", name="gk")
            nc.sync.dma_start(gk, gk_i)
            bk = sb.tile([P, NH], F32, tag="bk", name="bk")
            nc.sync.dma_start(bk, bk_i)

            proj_ctx = contextlib.ExitStack()
            proj = proj_ctx.enter_context(tc.tile_pool(name="proj", bufs=1))

            def stats_finish(pstat):
                """pstat [1, QT] sumsq psum -> [P, QT] bf16 rsig broadcast."""
                lnv = proj.tile([1, QT], F32, tag="stats_sb", bufs=4,
                                name="lnv")
                nc.scalar.activation(lnv, pstat, AF.Ln, scale=1.0 / D,
                                     bias=eps1)
                rsig = proj.tile([1, QT], BF16, tag="stats_sb", bufs=4,
                                 name="rsig")
                nc.scalar.activation(rsig, lnv, AF.Exp, bias=zero1,
                                     scale=-0.5)
                ps_rep = psum.tile([P, QT], F32, tag="sc", bufs=2,
                                   name="ps_rep")
                nc.tensor.matmul(ps_rep, lhsT=ones_row, rhs=rsig)
                rsb = proj.tile([P, QT], BF16, tag="rsb", bufs=4, name="rsb")
                nc.vector.tensor_copy(rsb, ps_rep)
                return rsb

            def ln_apply(sl, rsb, g_sb, b_sb, ec):
                """In-place LN affine on a [P, QT] bf16 slab slice."""
                nc.vector.tensor_tensor(sl, sl, rsb, op=OP.mult)
                nc.vector.tensor_scalar(
                    sl, sl,
                    scalar1=g_sb[:, ds(ec, 1)],
                    scalar2=b_sb[:, ds(ec, 1)],
                    op0=OP.mult, op1=OP.add,
                )

            def rope_apply(sl, cos_sl, sin_sl):
                """In-place rope on a [P, QT] bf16 slab slice."""
                ps_rot = psum.tile([P, QT], F32, tag="sc", bufs=2,
                                   name="ps_rot")
                nc.tensor.matmul(ps_rot, lhsT=rotm, rhs=sl)
                tmp = proj.tile([P, QT], BF16, tag="tmp", bufs=2, name="tmp")
                nc.vector.tensor_tensor(tmp, ps_rot, sin_sl, op=OP.mult)
                nc.vector.tensor_tensor(sl, sl, cos_sl, op=OP.mult)
                nc.vector.tensor_tensor(sl, sl, tmp, op=OP.add)

            # ---------------- emission helpers -----------------------
            # Deferred-work queues: each entry is a closure emitting the
            # LN+rope (DVE + 1 small PE matmul) for one head chunk. They
            # are drained interleaved into the next phase's matmul stream
            # so the PE never stalls on DVE and the DVE FIFO never blocks
            # a PSUM-slot reuse.
            def kproj_pair(pr, drain):
                """k projection for 1024-token pair pr; returns (rsbs,
                deferred ln+rope closures). drain: list of closures to
                interleave (one per ec)."""
                tok0 = pr * 1024
                kslab = kslabs[pr]
                xb = proj.tile([P, DC, 1024], BF16, tag="xb", bufs=1,
                               name="xb")
                for d in range(DC):
                    nc.sync.dma_start(xb[:, d],
                                      xT[ds(d * P, P), ds(tok0, 1024)])
                cosk = proj.tile([HD, 1024], BF16, tag="cs2", bufs=4,
                                 name="cosk")
                nc.sync.dma_start(cosk, cosk_i[:, ds(tok0, 1024)])
                sink = proj.tile([HD, 1024], BF16, tag="cs2", bufs=4,
                                 name="sink")
                nc.sync.dma_start(sink, sink_i[:, ds(tok0, 1024)])
                pstats = [
                    psum.tile([1, QT], F32, tag="st", bufs=2, name="pstat_k")
                    for _ in range(2)
                ]
                for ec in range(NH):
                    w = proj.tile([P, DC, P], BF16, tag="w", bufs=3,
                                  name="wk")
                    nc.sync.dma_start(w, wqkT[NH + ec])
                    psk = psum.tile([P, 1024], F32, tag="pair", bufs=2,
                                    name="psk")
                    for d in range(DC):
                        for half in range(2):
                            nc.tensor.matmul(
                                psk[:, ds(half * QT, QT)], lhsT=w[:, d],
                                rhs=xb[:, d, ds(half * QT, QT)],
                                start=(d == 0), stop=(d == DC - 1),
                            )
                    nc.vector.tensor_copy(kslab[:, ec, :], psk)
                    sq = proj.tile([P, 1024], BF16, tag="sq", bufs=2,
                                   name="sqk")
                    nc.scalar.square(sq, psk)
                    for half in range(2):
                        nc.tensor.matmul(pstats[half], lhsT=ones_col,
                                         rhs=sq[:, ds(half * QT, QT)],
                                         start=(ec == 0), stop=(ec == NH - 1))
                    if ec < len(drain):
                        drain[ec]()
                rsbs = [stats_finish(pstats[half]) for half in range(2)]
                return xb, rsbs, cosk, sink

            def k_lnrope_closures(pr, rsbs, cosk, sink):
                kslab = kslabs[pr]
                out = []
                for ec in range(NH):
                    def fn(ec=ec):
                        for half in range(2):
                            sl = kslab[:, ec, ds(half * QT, QT)]
                            ln_apply(sl, rsbs[half], gk, bk, ec)
                            rope_apply(sl, cosk[:, ds(half * QT, QT)],
                                       sink[:, ds(half * QT, QT)])
                    out.append(fn)
                return out

            def vproj_pair(pr, xb, drain):
                """v projection for pair pr using resident xb; writes vts
                (token-major) via per-head feature-major matmuls + PE block
                transposes. drain: closures interleaved per head."""
                for ecv in range(NH):
                    wvc = proj.tile([P, DC, P], BF16, tag="w", bufs=3,
                                    name="wvc")
                    nc.sync.dma_start(wvc, wvTt[ecv])
                    psvf = psum.tile([P, 1024], F32, tag="pair", bufs=2,
                                     name="psvf")
                    for d in range(DC):
                        for half in range(2):
                            nc.tensor.matmul(
                                psvf[:, ds(half * QT, QT)], lhsT=wvc[:, d],
                                rhs=xb[:, d, ds(half * QT, QT)],
                                start=(d == 0), stop=(d == DC - 1),
                            )
                    vfsb = proj.tile([P, 1024], BF16, tag="vsb", bufs=3,
                                     name="vfsb")
                    nc.vector.tensor_copy(vfsb, psvf)
                    vtsb = proj.tile([P, 8, HD], BF16, tag="vtsb", bufs=2,
                                     name="vtsb")
                    for tc8 in range(8):
                        pst = psum.tile([P, P], BF16, tag="st", bufs=2,
                                        name="pst")
                        nc.tensor.transpose(pst, vfsb[:, ds(tc8 * P, P)],
                                            ident)
                        nc.vector.tensor_copy(vtsb[:, tc8], pst)
                    nc.sync.dma_start(
                        vts[ecv, ds(pr * 1024, 1024), :].rearrange(
                            "(kc p) hd -> p kc hd", p=P
                        ),
                        vtsb,
                    )
                    if ecv < len(drain):
                        drain[ecv]()

            def qproj_tile(t, drain):
                """q projection for 512-token tile t into holdq; stats
                finished; returns (holdq, rsb)."""
                xq = proj.tile([P, DC, QT], BF16, tag="xq", bufs=1,
                               name="xq")
                for d in range(DC):
                    nc.sync.dma_start(xq[:, d],
                                      xTq[ds(d * P, P), ds(t * QT, QT)])
                cosq = proj.tile([HD, QT], BF16, tag="cs", bufs=4,
                                 name="cosq")
                nc.sync.dma_start(cosq, cosq_i[:, ds(t * QT, QT)])
                sinq = proj.tile([HD, QT], BF16, tag="cs", bufs=4,
                                 name="sinq")
                nc.sync.dma_start(sinq, sinq_i[:, ds(t * QT, QT)])
                holdq = proj.tile([P, NH, QT], BF16, tag="holdq", bufs=2,
                                  name="holdq")
                pstat = psum.tile([1, QT], F32, tag="st", bufs=2,
                                  name="pstat_q")
                for ec in range(NH):
                    w = proj.tile([P, DC, P], BF16, tag="w", bufs=3,
                                  name="wq")
                    nc.sync.dma_start(w, wqkT[ec])
                    psq = psum.tile([P, QT], F32, tag="sc", bufs=2,
                                    name="psq")
                    for d in range(DC):
                        nc.tensor.matmul(psq, lhsT=w[:, d], rhs=xq[:, d],
                                         start=(d == 0), stop=(d == DC - 1))
                    nc.vector.tensor_copy(holdq[:, ec], psq)
                    sq = proj.tile([P, QT], BF16, tag="sq", bufs=2,
                                   name="sqq")
                    nc.scalar.square(sq, psq)
                    nc.tensor.matmul(pstat, lhsT=ones_col, rhs=sq,
                                     start=(ec == 0), stop=(ec == NH - 1))
                    if ec < len(drain):
                        drain[ec]()
                rsb = stats_finish(pstat)
                return holdq, rsb, cosq, sinq

            def q_lnrope_closures(t, holdq, rsb, cosq, sinq):
                out = []
                for ec in range(NH):
                    def fn(ec=ec, last=(ec == NH - 1)):
                        sl = holdq[:, ec]
                        ln_apply(sl, rsb, gq, bq, ec)
                        rope_apply(sl, cosq, sinq)
                        if last:
                            nc.sync.dma_start(
                                qts[t].rearrange("h p q -> p h q"), holdq
                            )
                    out.append(fn)
                return out

            # ---------------- projection schedule ---------------------
            # each phase's LN/rope DVE work drains interleaved into the
            # next phase's matmul stream
            hq0, rsbq0, cq0, sq0 = qproj_tile(0, drain=[])
            lnq0 = q_lnrope_closures(0, hq0, rsbq0, cq0, sq0)
            xb0, rsbs0, ck0, sk0 = kproj_pair(0, drain=lnq0)
            ln0 = k_lnrope_closures(0, rsbs0, ck0, sk0)
            vproj_pair(0, xb0, drain=ln0)

            hq1, rsbq1, cq1, sq1 = qproj_tile(1, drain=[])
            lnq1 = q_lnrope_closures(1, hq1, rsbq1, cq1, sq1)
            xb1, rsbs1, ck1, sk1 = kproj_pair(1, drain=lnq1)
            ln1 = k_lnrope_closures(1, rsbs1, ck1, sk1)
            vproj_pair(1, xb1, drain=ln1)

            proj_ctx.close()

            # ============ attention + out-projection ===========
            attn_ctx = contextlib.ExitStack()
            attn = attn_ctx.enter_context(tc.tile_pool(name="attn", bufs=1))
            ot = attn.tile([P, NH, NQTOK], BF16, tag="ot", name="ot")
            PRE = 2  # score pair lookahead

            for t in range(NQ):
                mt = attn.tile([P, MAXM, QT], BF16, tag="masks", bufs=2,
                               name="mt")
                nc.sync.dma_start(mt, masks_i[t])
                mpos = {kc: i for i, kc in enumerate(masked[t])}
                n_slots = slots[t]
                n_pairs = n_slots // 2
                for h in range(NH):
                    qsl = attn.tile([P, QT], BF16, tag="qslab", bufs=3,
                                    name="qsl")
                    nc.sync.dma_start(qsl, qts[t, h])
                    vsl = attn.tile([P, KC, HD], BF16, tag="vslab", bufs=3,
                                    name="vsl")
                    nc.sync.dma_start(
                        vsl[:, :n_slots],
                        vts[h, ds(0, n_slots * P), :].rearrange(
                            "(kc p) hd -> p kc hd", p=P
                        ),
                    )
                    psout = psum.tile([P, QT], F32, tag="sc", bufs=2,
                                      name="psout")
                    dacc = attn.tile([P, 1024], BF16, tag="dacc", bufs=2,
                                     name="dacc")

                    ps_pairs = {}

                    def issue_scores(p, h=h, qsl=qsl, ps_pairs=ps_pairs):
                        pp = psum.tile([P, 1024], F32, tag="pair", bufs=2,
                                       name="pp")
                        for half in range(2):
                            kc = 2 * p + half
                            nc.tensor.matmul(
                                pp[:, ds(half * QT, QT)],
                                lhsT=kslabs[kc // 8][:, h,
                                                    ds((kc % 8) * P, P)],
                                rhs=qsl,
                            )
                        ps_pairs[p] = pp

                    for p in range(min(PRE, n_pairs)):
                        issue_scores(p)
                    for p in range(n_pairs):
                        if p + PRE < n_pairs:
                            issue_scores(p + PRE)
                        pp = ps_pairs.pop(p)
                        et2 = attn.tile([P, 1024], BF16, tag="exp", bufs=3,
                                        name="et2")
                        nc.scalar.activation(et2, pp, AF.Exp, bias=nege)
                        for half in range(2):
                            kc = 2 * p + half
                            if kc in mpos:
                                nc.vector.tensor_tensor(
                                    et2[:, ds(half * QT, QT)],
                                    et2[:, ds(half * QT, QT)],
                                    mt[:, mpos[kc]], op=OP.mult,
                                )
                        for half in range(2):
                            kc = 2 * p + half
                            nc.tensor.matmul(
                                psout,
                                lhsT=vsl[:, kc],
                                rhs=et2[:, ds(half * QT, QT)],
                                start=(kc == 0), stop=(kc == n_slots - 1),
                            )
                        # denominator accumulates on DVE, off the PE
                        if p == 0:
                            nc.vector.tensor_copy(dacc, et2)
                        else:
                            nc.vector.tensor_tensor(dacc, dacc, et2,
                                                    op=OP.add)

                    psden = psum.tile([1, QT], F32, tag="st", bufs=2,
                                      name="psden")
                    for half in range(2):
                        nc.tensor.matmul(psden, lhsT=ones_col,
                                         rhs=dacc[:, ds(half * QT, QT)],
                                         start=(half == 0), stop=(half == 1))
                    rec0 = attn.tile([1, QT], F32, tag="stats_sb", bufs=4,
                                     name="rec0")
                    with nc.allow_low_precision(
                        reason="denominator reciprocal, 18 bits is plenty"
                    ):
                        nc.vector.reciprocal_approx_fast(rec0, psden)
                    rec = attn.tile([1, QT], BF16, tag="stats_sb", bufs=4,
                                    name="rec")
                    nc.scalar.activation(rec, rec0, AF.Copy)
                    psr = psum.tile([P, QT], F32, tag="sc", bufs=2,
                                    name="psr")
                    nc.tensor.matmul(psr, lhsT=ones_row, rhs=rec)
                    rsb = attn.tile([P, QT], BF16, tag="rsbn", bufs=2,
                                    name="rsbn")
                    nc.vector.tensor_copy(rsb, psr)
                    nc.vector.tensor_tensor(ot[:, h, ds(t * QT, QT)], psout,
                                            rsb, op=OP.mult)

            # ---- out-projection over all q tokens ----
            for ec in range(DC):
                woc = attn.tile([P, NH, P], BF16, tag="woc", bufs=3,
                                name="woc")
                nc.sync.dma_start(woc, woTt[ec])
                psf = psum.tile([P, 1024], F32, tag="pair", bufs=2,
                                name="psf")
                for h in range(NH):
                    for half in range(2):
                        nc.tensor.matmul(
                            psf[:, ds(half * QT, QT)], lhsT=woc[:, h],
                            rhs=ot[:, h, ds(half * QT, QT)],
                            start=(h == 0), stop=(h == NH - 1),
                        )
                fsb = attn.tile([P, 1024], F32, tag="fsb", bufs=2, name="fsb")
                nc.vector.tensor_copy(fsb, psf)
                nc.sync.dma_start(out_t[ds(ec * P, P), :], fsb)

            attn_ctx.close()

    nc.compile()
    return nc


# --------------------------------------------------------------------------
# Host-side prep and driver
# --------------------------------------------------------------------------

def _q_blocks(role):
    """q-block indices (each 512 tokens) for a core role."""
    return [0, 3] if role == 0 else [1, 2]


def make_host_data(x, w_in, w_out, q_gamma, q_beta, k_gamma, k_beta, cfg,
                   n_cores=None):
    """Build per-core in_maps (list of dicts) + assembly metadata."""
    import ml_dtypes

    BF = ml_dtypes.bfloat16
    D = cfg["D"]
    S = cfg["S"]
    NQTOK = cfg["NQTOK"]
    QT = cfg["QT"]
    masked = cfg["masked"]
    NH = D // HD
    DC = D // P
    NQ = NQTOK // QT
    MAXM = max(len(m) for m in masked)
    B = x.shape[0]
    if n_cores is None:
        n_cores = 2 * B

    w64 = np.asarray(w_in, np.float64)
    wq = w64[0:D]
    wk = w64[D:2 * D]
    wv = w64[2 * D:3 * D]
    wq_c = wq - wq.mean(axis=0, keepdims=True)
    wk_c = wk - wk.mean(axis=0, keepdims=True)
    wqkT2 = np.concatenate([wq_c.T, wk_c.T], axis=1).astype(BF)
    # pre-tile to [2*NH, P, DC, P]: tile ec -> [p, dc, e] with contiguous rows
    wqkT = np.ascontiguousarray(
        wqkT2.reshape(DC, P, 2 * NH, P).transpose(2, 1, 0, 3)
    )
    wvTt = np.ascontiguousarray(
        wv.T.astype(BF).reshape(DC, P, NH, P).transpose(2, 1, 0, 3)
    )
    woT = np.asarray(w_out, np.float64).T  # [feat, e]
    # [ec, p(feat within head), h, e-col]
    woTt = np.ascontiguousarray(
        woT.reshape(NH, P, DC, P).transpose(2, 1, 0, 3).astype(BF)
    )

    inv = 1.0 / (10000.0 ** (np.arange(0, HD, 2, dtype=np.float64) / HD))
    tpos = np.arange(S, dtype=np.float64)
    fr = np.outer(tpos, inv)
    emb = np.concatenate([fr, fr], axis=-1)  # [S, HD]
    cosT = np.cos(emb).T  # [HD, S]
    sinTn = np.sin(emb).T

    # signed rotate-half permutation, as matmul lhsT:
    # out[p] = sum_{p'} rotmT[p', p] * in[p'] = rot_half(in)[p]
    h2 = HD // 2
    rotmT = np.zeros((P, P), np.float32)
    for p in range(h2):
        rotmT[p + h2, p] = -1.0
    for p in range(h2, HD):
        rotmT[p - h2, p] = 1.0

    scale = 1.0 / math.sqrt(HD)
    gq_a = np.ascontiguousarray(
        (np.asarray(q_gamma, np.float64) * scale).reshape(NH, P).T
    ).astype(np.float32)
    bq_a = np.ascontiguousarray(
        (np.asarray(q_beta, np.float64) * scale).reshape(NH, P).T
    ).astype(np.float32)
    gk_a = np.ascontiguousarray(
        np.asarray(k_gamma, np.float32).reshape(NH, P).T
    )
    bk_a = np.ascontiguousarray(
        np.asarray(k_beta, np.float32).reshape(NH, P).T
    )

    cosk = np.ascontiguousarray(cosT.astype(BF))
    sink = np.ascontiguousarray(sinTn.astype(BF))

    in_maps = []
    meta = []
    cores_per_batch = max(1, n_cores // B)
    for c in range(n_cores):
        b = c // cores_per_batch
        r = c % cores_per_batch
        blocks = _q_blocks(r)
        qtok = np.concatenate(
            [np.arange(bk * 512, (bk + 1) * 512) for bk in blocks]
        )
        xb = np.asarray(x[b], np.float32)  # [S, D]
        xTf = xb.T.astype(BF)              # [D, S]
        xT = np.ascontiguousarray(xTf)
        xTq = np.ascontiguousarray(xTf[:, qtok])
        cosq = np.ascontiguousarray(cosT[:, qtok].astype(BF))
        sinq = np.ascontiguousarray(sinTn[:, qtok].astype(BF))

        masks = np.zeros([NQ, P, MAXM, QT], np.float32)
        for t in range(NQ):
            q_start = blocks[t] * 512
            qq = np.arange(QT)
            kk = np.arange(P)
            for mi, kc in enumerate(masked[t]):
                masks[t, :, mi, :] = (
                    (kc * P + kk[:, None]) <= (q_start + qq[None, :])
                ).astype(np.float32)
        masks = masks.astype(BF)

        in_maps.append(dict(
            xTq=xTq, xT=xT, wqkT=wqkT, wvTt=wvTt, woTt=woTt,
            cosq=cosq, sinqn=sinq, cosk=cosk, sinkn=sink,
            gq=gq_a, bq=bq_a, gk=gk_a, bk=bk_a, masks=masks,
            rotm=rotmT.astype(BF),
            ident=np.eye(P, dtype=np.float32).astype(BF),
            onesc=np.ones((P, 1), BF),
            onesr=np.ones((1, P), BF),
        ))
        meta.append(dict(b=b, qtok=qtok))
    return in_maps, meta


_PROGRAM_CACHE = {}


def _get_program(cfg_key, cfg):
    if cfg_key not in _PROGRAM_CACHE:
        _PROGRAM_CACHE[cfg_key] = build_program(cfg)
    return _PROGRAM_CACHE[cfg_key]


def run_full(x, w_in, w_out, q_gamma, q_beta, k_gamma, k_beta,
             trace=False):
    from concourse.bass_utils import run_bass_kernel_spmd

    cfg = FULL_CFG
    B = x.shape[0]
    n_cores = 2 * B
    in_maps, meta = make_host_data(
        x, w_in, w_out, q_gamma, q_beta, k_gamma, k_beta, cfg,
        n_cores=n_cores,
    )
    nc = _get_program("full", cfg)
    res = run_bass_kernel_spmd(
        nc, in_maps, core_ids=list(range(n_cores)), trace=trace,
    )
    S, D = cfg["S"], cfg["D"]
    out = np.empty((B, S, D), np.float32)
    for c in range(n_cores):
        o = res.results[c]["out"]  # [D, NQTOK]
        out[meta[c]["b"], meta[c]["qtok"], :] = o.T
    return out, res


def kernel(x, w_in, w_out, q_gamma, q_beta, k_gamma, k_beta, n_heads=16,
           **_ignored):
    x = np.asarray(x, np.float32)
    assert int(np.asarray(n_heads)) * HD == x.shape[-1]
    out, _ = run_full(
        np.asarray(x, np.float32),
        np.asarray(w_in, np.float32),
        np.asarray(w_out, np.float32),
        np.asarray(q_gamma, np.float32),
        np.asarray(q_beta, np.float32),
        np.asarray(k_gamma, np.float32),
        np.asarray(k_beta, np.float32),
    )
    return out
